# revision 16
# baseline (speedup 1.0000x reference)
"""BatchHardLoss on 8 Trainium2 NeuronCores (Bass/Tile).

loss = mean_i log( pos_sum_i * neg_sum_i )
  W = clip(gamma * X @ X.T, -16, 16)   [B, B]
  pos_sum_i = sum_{j: t_j == t_i, j != i} exp(-W_ij)
  neg_sum_i = sum_{j: t_j != t_i} exp(+W_ij)

Strategy (v3, symmetric + lagged column sums):
- Host sorts rows by class; same-class columns then sit in a narrow
  window per 128-row tile (pos/negcorr handled by a masked window pass).
- Rows sharded: core c owns the 1024 sorted rows [1024c, 1024c+1024).
- exp(W) is symmetric: the full-matrix row sums S_i come from a 33-tile
  circulant band per row tile (own block + distances d=1..32).  Each
  exp'd block feeds its row accumulator (ACT accum_out) and its mirror
  column accumulator (ones-matmul column sums on PE).  The d=32 block is
  halved (ACT bias -ln2) since both mirror tiles compute it.
- Column-sum matmuls for tile t are emitted during tile t+1's matmul
  stream so PE never stalls waiting for tile t's ACT outputs.
- SPMD uniformity: each core's columns are rotated so its own rows sit
  at local column 0; the band is then the same static slice pattern on
  every core.  Host un-rotates/sums column accumulators and finishes
  log + mean.
- "aligned" fast path (the expected balanced-classes case): every
  tile's same-class columns lie inside its own diagonal 128-block, so
  the window pass reads the diag part of the g0 PSUM directly (no xwin
  input, no extra matmuls).
- gamma*|dot| <= ~0.4 << 16 for this data (checked), so the clip is a
  no-op.
"""

import math

import numpy as np
import ml_dtypes

B = 8192
D = 256
GAMMA = 0.001
NCORES = 8
P = 128                      # partitions / rows per tile
TILES = 8                    # row tiles per core (1024 rows/core)
NTILES = B // P              # 64 global tiles
ROWS_PER_CORE = P * TILES
KCH = 2                      # contraction chunks (D = 2*128)
BAND = 32                    # column-tile distances 1..BAND
GROUP = 1536                 # band columns per PSUM group (3 banks)
MASKVAL = 49152.0            # exactly representable in bf16/f32
BIAS_POS = float(-(np.float32(GAMMA) * np.float32(MASKVAL)))
BIAS_HALF = -math.log(2.0)

_program_cache = {}

# band covers the tile's own block + d=1..32: 33*128 = 4224 columns,
# grouped into PSUM groups of <= GROUP columns; the final 128 columns
# (the d=32 block) get a halved exp.
def _band_groups():
    groups = []
    total = (BAND + 1) * P   # 4224
    pos = 0
    while pos < total:
        w = min(GROUP, total - pos)
        groups.append((pos, w, [(0, w, False)]))
        pos += w
    return groups


def _build_program(cw, aligned):
    import concourse.bacc as bacc
    import concourse.tile as tile
    from concourse import mybir

    dt = mybir.dt
    Exp = mybir.ActivationFunctionType.Exp
    sub = mybir.AluOpType.subtract
    add = mybir.AluOpType.add
    mult = mybir.AluOpType.mult
    DR = mybir.MatmulPerfMode.DoubleRow

    nc = bacc.Bacc("TRN2", target_bir_lowering=False, debug=False,
                   num_devices=NCORES)

    xfull = nc.declare_dram_parameter("xfull", [P, KCH, B], dt.float8e4, isOutput=False)
    if not aligned:
        xwin = nc.declare_dram_parameter("xwin", [P, TILES, KCH, cw], dt.float8e4, isOutput=False)
    posm = nc.declare_dram_parameter("posm", [P, TILES, cw], dt.bfloat16, isOutput=False)
    negm = nc.declare_dram_parameter("negm", [P, TILES, cw], dt.bfloat16, isOutput=False)
    rows_out = nc.declare_dram_parameter("rows_out", [P, TILES], dt.float32, isOutput=True)
    possum_out = nc.declare_dram_parameter("possum_out", [P, TILES], dt.float32, isOutput=True)
    negcorr_out = nc.declare_dram_parameter("negcorr_out", [P, TILES], dt.float32, isOutput=True)
    colacc_out = nc.declare_dram_parameter("colacc_out", [P, NTILES], dt.float32, isOutput=True)

    groups = _band_groups()
    nparts = sum(len(a) for _, _, a in groups)

    with tile.TileContext(nc) as tc:
        with (
            tc.tile_pool(name="resident", bufs=1) as resident,
            tc.tile_pool(name="psum", bufs=2, space="PSUM") as psum_pool,
            tc.tile_pool(name="cpsum", bufs=1, space="PSUM") as cpsum_pool,
            tc.tile_pool(name="escratch", bufs=6) as escratch,
            tc.tile_pool(name="scratch", bufs=2) as scratch,
            tc.tile_pool(name="acc", bufs=1) as acc,
        ):
            xfull_sb = resident.tile([P, KCH, B], dt.float8e4)
            posm_sb = resident.tile([P, TILES, cw], dt.bfloat16)
            negm_sb = resident.tile([P, TILES, cw], dt.bfloat16)

            # band columns for early tiles first
            nc.sync.dma_start(out=xfull_sb[:, :, 0:1536], in_=xfull[:, :, 0:1536])
            nc.sync.dma_start(out=xfull_sb[:, :, 1536:3072], in_=xfull[:, :, 1536:3072])
            nc.sync.dma_start(out=xfull_sb[:, :, 3072:5248], in_=xfull[:, :, 3072:5248])
            nc.sync.dma_start(out=xfull_sb[:, :, 5248:B], in_=xfull[:, :, 5248:B])
            if not aligned:
                xwin_sb = resident.tile([P, TILES, KCH, cw], dt.float8e4)
                nc.gpsimd.dma_start(out=xwin_sb[:], in_=xwin[:])
            nc.gpsimd.dma_start(out=posm_sb[:], in_=posm[:])
            nc.gpsimd.dma_start(out=negm_sb[:], in_=negm[:])

            bias_pos = acc.tile([P, 1], dt.float32)
            nc.vector.memset(bias_pos[:], BIAS_POS)
            ones_bf = acc.tile([P, 1], dt.bfloat16)
            nc.vector.memset(ones_bf[:], 1.0)
            zeros_bf = acc.tile([P, P], dt.bfloat16)
            nc.vector.memset(zeros_bf[:], 0.0)

            rowparts = acc.tile([P, TILES, nparts], dt.float32)
            possum = acc.tile([P, TILES], dt.float32)
            negcorr = acc.tile([P, TILES], dt.float32)
            colacc_ps = cpsum_pool.tile([P, NTILES], dt.float32)
            # start=True clears has_written for the WHOLE bank, so it may
            # only ever happen once on this bank: zero all slots up front
            # (setting every element's has_written), then pure-accumulate.
            nc.tensor.matmul(
                colacc_ps[:, 0:NTILES],
                lhsT=zeros_bf[:, 0:P],
                rhs=zeros_bf[:, 0:NTILES],
                start=True, stop=False, skip_group_check=True,
            )

            # tile t's colsum work, deferred into tile t+1's stream:
            # list of (esb, local sub offset, jt slot)
            pending = []

            def flush_pending(final):
                for (esb, soff, jt, last) in pending:
                    nc.tensor.matmul(
                        colacc_ps[:, jt:jt + 1],
                        lhsT=esb[:, soff:soff + P],
                        rhs=ones_bf[:, 0:1],
                        start=False,
                        stop=(last and final),
                        skip_group_check=True,
                    )
                pending.clear()

            for t in range(TILES):
                r0 = t * P
                slot = 0
                tile_pend = []
                for gi, (g0, gw, acts) in enumerate(groups):
                    ps = psum_pool.tile([P, GROUP], dt.float32, tag="big")
                    for p0 in range(0, gw, 512):
                        p1 = min(p0 + 512, gw)
                        c0 = r0 + g0 + p0
                        nc.tensor.matmul(
                            ps[:, p0:p1],
                            lhsT=xfull_sb[:, :, r0:r0 + P],
                            rhs=xfull_sb[:, :, c0:c0 + (p1 - p0)],
                            start=True, stop=True, perf_mode=DR,
                        )
                    esb = escratch.tile([P, GROUP], dt.bfloat16, tag="E")
                    for (a0, aw, halved) in acts:
                        nc.scalar.activation(
                            esb[:, a0:a0 + aw], ps[:, a0:a0 + aw], Exp,
                            scale=GAMMA,
                            accum_out=rowparts[:, t, slot:slot + 1],
                        )
                        slot += 1
                    for csub in range(gw // P):
                        d = (g0 // P) + csub      # distance 0..32
                        if d == 0 or d == BAND:
                            # diag: row-only.  d=32: both mirror tiles
                            # compute it row-side in full, so no colsum.
                            continue
                        jt = t + d
                        tile_pend.append((esb, csub * P, jt, d == BAND - 1))

                    if gi == 0:
                        # window pass: pos/neg same-class sums from the E
                        # diag block via DVE (reciprocal for exp(-W)).
                        if aligned:
                            ewin = esb[:, 0:cw]
                        else:
                            pw = psum_pool.tile([P, GROUP], dt.float32, tag="big")
                            for m0 in range(0, cw, 512):
                                m1 = min(m0 + 512, cw)
                                nc.tensor.matmul(
                                    pw[:, m0:m1],
                                    lhsT=xfull_sb[:, :, r0:r0 + P],
                                    rhs=xwin_sb[:, t, :, m0:m1],
                                    start=True, stop=True, perf_mode=DR,
                                )
                            ewsb = scratch.tile([P, cw], dt.bfloat16, tag="ew")
                            nc.scalar.activation(
                                ewsb[:], pw[:, 0:cw], Exp, scale=GAMMA)
                            ewin = ewsb[:]
                        nmasked = scratch.tile([P, cw], dt.float32, tag="wpre")
                        nc.vector.tensor_tensor(
                            out=nmasked[:], in0=ewin, in1=negm_sb[:, t, :], op=mult)
                        nc.vector.reduce_sum(
                            negcorr[:, t:t + 1], nmasked[:],
                            axis=mybir.AxisListType.X)
                        recip = scratch.tile([P, cw], dt.float32, tag="wrec")
                        nc.vector.reciprocal(recip[:], ewin)
                        pmasked = scratch.tile([P, cw], dt.float32, tag="wpre")
                        nc.vector.tensor_tensor(
                            out=pmasked[:], in0=recip[:], in1=posm_sb[:, t, :], op=mult)
                        nc.vector.reduce_sum(
                            possum[:, t:t + 1], pmasked[:],
                            axis=mybir.AxisListType.X)
                        # previous tile's colsums ride behind this tile's
                        # first matmul group
                        flush_pending(final=False)
                    elif t == TILES - 1:
                        # last tile: nothing rides behind it, so lag its
                        # colsums by one group instead of one tile
                        flush_pending(final=False)
                        pending = [p for p in tile_pend if p[0] is esb]
                        tile_pend = [p for p in tile_pend if p[0] is not esb]
                pending = tile_pend + pending if t == TILES - 1 else tile_pend
            flush_pending(final=True)

            # ---- wrap up ----
            rowsum = acc.tile([P, TILES], dt.float32)
            nc.vector.reduce_sum(
                rowsum[:, :], rowparts[:, :, :], axis=mybir.AxisListType.X)
            colacc_sb = acc.tile([P, NTILES], dt.float32)
            nc.vector.tensor_copy(colacc_sb[:], colacc_ps[:])
            nc.sync.dma_start(out=rows_out[:], in_=rowsum[:])
            nc.sync.dma_start(out=possum_out[:], in_=possum[:])
            nc.sync.dma_start(out=negcorr_out[:], in_=negcorr[:])
            nc.sync.dma_start(out=colacc_out[:], in_=colacc_sb[:])

    nc.compile()
    return nc


def _numpy_fallback(x, t):
    x = x.astype(np.float32)
    total = 0.0
    for r0 in range(0, B, 1024):
        w = np.clip(x[r0:r0 + 1024] @ x.T * GAMMA, -16.0, 16.0)
        same = t[r0:r0 + 1024, None] == t[None, :]
        notself = np.ones_like(same)
        idx = np.arange(r0, r0 + 1024)
        notself[np.arange(1024), idx] = False
        pos = same & notself
        pos_sum = np.where(pos, np.exp(-w), 0.0).sum(axis=1)
        neg_sum = np.where(~same, np.exp(w), 0.0).sum(axis=1)
        total += np.log(pos_sum * neg_sum).sum(dtype=np.float64)
    return np.float32(total / B)


def kernel(inputs, targets):
    from concourse.bass_utils import run_bass_kernel_spmd

    x = np.asarray(inputs, dtype=np.float32)
    t = np.asarray(targets, dtype=np.int32)
    assert x.shape == (B, D) and t.shape == (B,)

    order = np.argsort(t, kind="stable")
    ts = t[order]
    xs = x[order]

    # the clip in the reference must be a no-op for our mask algebra
    max_norm2 = float((xs.astype(np.float64) ** 2).sum(axis=1).max())
    if GAMMA * max_norm2 > 8.0:
        return _numpy_fallback(x, t)

    # class windows per 128-row tile (sorted order)
    cls_start = np.searchsorted(ts, ts, side="left")
    cls_end = np.searchsorted(ts, ts, side="right")
    wins = []
    need = 0
    aligned = True
    for r0 in range(0, B, P):
        w0 = int(cls_start[r0])
        w1 = int(cls_end[r0 + P - 1])
        need = max(need, w1 - w0)
        if w0 < r0 or w1 > r0 + P:
            aligned = False
        wins.append((w0, w1))
    if aligned:
        cw = P
    else:
        cw = max(256, ((need + 127) // 128) * 128)
        if cw > 1024:
            return _numpy_fallback(x, t)

    xs_q = xs.astype(ml_dtypes.float8_e4m3)
    XT = np.ascontiguousarray(xs_q.T)                      # [256, 8192]
    xfull_g = np.ascontiguousarray(
        XT.reshape(KCH, P, B).transpose(1, 0, 2))          # [128, 2, 8192]

    in_maps = []
    for c in range(NCORES):
        lo = c * ROWS_PER_CORE
        xfull_c = np.ascontiguousarray(
            np.concatenate([xfull_g[:, :, lo:], xfull_g[:, :, :lo]], axis=2))
        posm_t = np.empty((P, TILES, cw), dtype=ml_dtypes.bfloat16)
        negm_t = np.empty((P, TILES, cw), dtype=ml_dtypes.bfloat16)
        if not aligned:
            xwin_t = np.empty((P, TILES, KCH, cw), dtype=ml_dtypes.float8_e4m3)
        for ti in range(TILES):
            r0 = lo + ti * P
            if aligned:
                w = r0
            else:
                w0, w1 = wins[r0 // P]
                w = min(w0, B - cw)
                assert w1 - w <= cw
                xwin_t[:, ti] = XT[:, w:w + cw].reshape(KCH, P, cw).transpose(1, 0, 2)
            rows_t = ts[r0:r0 + P]
            cols_t = ts[w:w + cw]
            same = rows_t[:, None] == cols_t[None, :]
            colidx = np.arange(w, w + cw)[None, :]
            rowidx = np.arange(r0, r0 + P)[:, None]
            pos = same & (colidx != rowidx)
            posm_t[:, ti] = pos.astype(ml_dtypes.bfloat16)
            negm_t[:, ti] = same.astype(ml_dtypes.bfloat16)
        im = {"xfull": xfull_c, "posm": posm_t, "negm": negm_t}
        if not aligned:
            im["xwin"] = xwin_t
        in_maps.append(im)

    key = (cw, aligned)
    if key not in _program_cache:
        _program_cache[key] = _build_program(cw, aligned)
    nc = _program_cache[key]

    res = run_bass_kernel_spmd(nc, in_maps, core_ids=list(range(NCORES)))

    # host combine: S_i = rowS_i + colacc_i  (column sums un-rotated)
    colglob = np.zeros((P, NTILES), dtype=np.float64)
    for c in range(NCORES):
        ca = res.results[c]["colacc_out"].astype(np.float64)
        for jt in range(1, TILES + BAND - 1):
            colglob[:, (jt + TILES * c) % NTILES] += ca[:, jt]
    S = np.empty((P, NTILES), dtype=np.float64)
    possum = np.empty((P, NTILES), dtype=np.float64)
    negcorr = np.empty((P, NTILES), dtype=np.float64)
    for c in range(NCORES):
        sl = slice(c * TILES, (c + 1) * TILES)
        S[:, sl] = res.results[c]["rows_out"].astype(np.float64)
        possum[:, sl] = res.results[c]["possum_out"].astype(np.float64)
        negcorr[:, sl] = res.results[c]["negcorr_out"].astype(np.float64)
    S += colglob
    per_row = np.log(possum * (S - negcorr))
    return np.float32(per_row.mean())
